# revision 27
# baseline (speedup 1.0000x reference)
"""Causal self-attention (B=2, T=2048, C=1024, H=16) on 8 trn2 NeuronCores.

Sharding: data-parallel over batch (2) x tensor-parallel over head groups (4);
each core handles one (batch, 4-head group) pair end to end: qkv projection
(column-sliced weights) -> causal attention -> proj with row-sliced w_proj
producing a partial [T, C] output; host sums 4 partials per batch + b_proj.

Device scheme (all matmul operands float32r; PSUM accumulates fp32):
  xt   [C, T]       x[b] transposed (host); moving operand for Q^T/K^T,
                    stationary for V.
  Q^T/K^T [128,2,T] head-dim on partitions, 2 heads per 128-partition tile;
                    softmax 1/sqrt(D) folded into wq/bq on host.
  V'   [128,16,4*65] per k-tile: 64 value cols + a ones column per head
                    (P@[V|1] row 64 = softmax denominator).
  S^T  [128k, 512q] scores in PSUM, kt-pairs share one [128,1024] tile so a
                    single exp covers two k-tiles; causal = block skip above
                    the diagonal + additive -1e30 mask on diagonal blocks;
                    no max subtraction (scores ~N(0,1)).
  y'^T [65, 512]    accumulated over k-tiles; normalized by the denominator
                    row via fast reciprocal + DRAM-bounce broadcast.

Perf structure: QKV runs kc-outer over 8 concurrent PSUM groups so matmuls
start as soon as the first xt/w chunks land; attention interleaves two head
streams (partitions 0-63 and 64-127) so the PE never idles while ACT does
exp (keeps the HAM clock gate warm); proj for chunk qc is emitted one qc
late so the normalization chain is off the critical path.
"""

import numpy as np

N_CORES = 8
B, T, C = 2, 2048, 1024
H, D = 16, 64
HL = 4            # heads per core
CL = HL * D       # 256 local qkv columns per core
KC = C // 128     # 8 contraction chunks
NT = T // 512     # 4 free-dim chunks
TT = T // 128     # 16 token tiles
NEG = -1.0e30


def build_bass():
    from contextlib import ExitStack

    import concourse.mybir as mybir
    import concourse.tile as tile
    from concourse import bacc

    f32 = mybir.dt.float32
    f32r = mybir.dt.float32r
    AF = mybir.ActivationFunctionType
    ALU = mybir.AluOpType

    nc = bacc.Bacc("TRN2", target_bir_lowering=False, debug=False)

    xt_d = nc.dram_tensor("xt", [C, T], f32, kind="ExternalInput").ap()
    wq_d = nc.dram_tensor("wq", [128, KC * CL], f32, kind="ExternalInput").ap()
    wk_d = nc.dram_tensor("wk", [128, KC * CL], f32, kind="ExternalInput").ap()
    wv_d = nc.dram_tensor("wv", [128, KC * CL], f32, kind="ExternalInput").ap()
    bq_d = nc.dram_tensor("bq", [2, 128], f32, kind="ExternalInput").ap()
    bk_d = nc.dram_tensor("bk", [2, 128], f32, kind="ExternalInput").ap()
    bv_d = nc.dram_tensor("bv", [CL], f32, kind="ExternalInput").ap()
    wp_d = nc.dram_tensor("wp", [128, 2 * C], f32, kind="ExternalInput").ap()
    cm_d = nc.dram_tensor("cmask", [128, 640], f32, kind="ExternalInput").ap()
    vones_d = nc.dram_tensor("vones", [1], f32, kind="ExternalInput").ap()
    out_d = nc.dram_tensor("out", [T, C], f32, kind="ExternalOutput").ap()

    with tile.TileContext(nc) as tc, ExitStack() as ctx:
        singles = ctx.enter_context(tc.tile_pool(name="singles", bufs=1))
        ptp = ctx.enter_context(tc.tile_pool(name="ptp", bufs=3))
        small = ctx.enter_context(tc.tile_pool(name="small", bufs=3))
        outp = ctx.enter_context(tc.tile_pool(name="outp", bufs=2))
        dscr = ctx.enter_context(tc.tile_pool(name="dscr", bufs=3, space="DRAM"))

        # small constants first, on the gpsimd (SWDGE) queue
        bq_sb = singles.tile([128, 2], f32)
        nc.gpsimd.dma_start(bq_sb[:], bq_d.rearrange("pt p -> p pt"))
        bk_sb = singles.tile([128, 2], f32)
        nc.gpsimd.dma_start(bk_sb[:], bk_d.rearrange("pt p -> p pt"))
        bv_sb = singles.tile([128, CL], f32)
        nc.gpsimd.dma_start(bv_sb[:], bv_d[None, :].to_broadcast([128, CL]))
        cm_sb = singles.tile([128, 640], f32)
        nc.gpsimd.dma_start(cm_sb[:], cm_d[:])

        wq_sb = singles.tile([128, KC, CL], f32r)
        wk_sb = singles.tile([128, KC, CL], f32r)
        wv_sb = singles.tile([128, KC, CL], f32r)
        wp_sb = singles.tile([128, 2, C], f32r)

        qt_sb = singles.tile([128, 2, T], f32r)
        kt_sb = singles.tile([128, 2, T], f32r)
        v_sb = singles.tile([128, TT, HL * 65], f32r)
        yt_sb = singles.tile([128, 2, T], f32r)

        # ones column per (k-tile, head) for the denominator trick
        v_ones = v_sb[:].rearrange("p t (h e) -> p (t h) e", e=65)[:, :, 64:65]
        nc.gpsimd.dma_start(
            v_ones,
            vones_d.bitcast(f32r)[None, None, :].to_broadcast([128, TT * HL, 1]),
        )

        # ---- Phase A+B: qkv projections (xt resident only here) ----
        with tc.tile_pool(name="xtp", bufs=1) as xtp:
            xt_sb = xtp.tile([128, KC, T], f32r)
            # per-chunk loads, interleaved across both HWDGE queues so the
            # kc-outer matmul passes can start after the first chunks land
            # all big input loads on the sync queue only: one HWDGE queue
            # reaches full rate, and the scalar queue must stay free for the
            # psum-evacuation ACTIVATEs that recycle the qkv psum slots
            for kc in range(KC):
                nc.sync.dma_start(
                    wq_sb[:, kc, :],
                    wq_d[:, kc * CL : (kc + 1) * CL].bitcast(f32r),
                )
                nc.sync.dma_start(
                    wk_sb[:, kc, :],
                    wk_d[:, kc * CL : (kc + 1) * CL].bitcast(f32r),
                )
                nc.sync.dma_start(
                    wv_sb[:, kc, :],
                    wv_d[:, kc * CL : (kc + 1) * CL].bitcast(f32r),
                )
                nc.sync.dma_start(
                    xt_sb[:, kc, :],
                    xt_d[kc * 128 : (kc + 1) * 128, :].bitcast(f32r),
                )
            nc.sync.dma_start(
                wp_sb[:], wp_d.rearrange("p (pc n) -> p pc n", pc=2).bitcast(f32r)
            )

            groups = [(pt, nt) for pt in range(2) for nt in range(NT)]
            with tc.tile_pool(name="ps8", bufs=8, space="PSUM") as ps8:
                for w_sb, b_sb, dst in (
                    (wq_sb, bq_sb, qt_sb),
                    (wk_sb, bk_sb, kt_sb),
                ):
                    tiles = [
                        ps8.tile([128, 512], f32, tag="qk", name=f"qkg{g}")
                        for g in range(len(groups))
                    ]
                    for kc in range(KC):
                        for g, (pt, nt) in enumerate(groups):
                            nc.tensor.matmul(
                                tiles[g][:],
                                w_sb[:, kc, pt * 128 : (pt + 1) * 128],
                                xt_sb[:, kc, nt * 512 : (nt + 1) * 512],
                                start=(kc == 0),
                                stop=(kc == KC - 1),
                            )
                    for g, (pt, nt) in enumerate(groups):
                        nc.scalar.activation(
                            out=dst[:, pt, nt * 512 : (nt + 1) * 512],
                            in_=tiles[g][:],
                            func=AF.Identity,
                            bias=b_sb[:, pt : pt + 1],
                            scale=1.0,
                        )

            with tc.tile_pool(name="ps_v", bufs=2, space="PSUM") as ps_v:
                for tt in range(TT):
                    ps = ps_v.tile([128, CL], f32, tag="v")
                    for kc in range(KC):
                        nc.tensor.matmul(
                            ps[:],
                            xt_sb[:, kc, tt * 128 : (tt + 1) * 128],
                            wv_sb[:, kc, :],
                            start=(kc == 0),
                            stop=(kc == KC - 1),
                        )
                    dst = v_sb[:, tt, :].rearrange("p (h e) -> p h e", e=65)[
                        :, :, 0:64
                    ]
                    nc.vector.tensor_tensor(
                        dst,
                        ps[:].rearrange("p (h e) -> p h e", e=64),
                        bv_sb[:].rearrange("p (h e) -> p h e", e=64),
                        ALU.add,
                    )

        # ---- Phase C+D: attention (two interleaved head streams) + proj ----
        with (
            tc.tile_pool(name="ps_s", bufs=3, space="PSUM") as ps_s,
            tc.tile_pool(name="ps_y", bufs=2, space="PSUM") as ps_y,
        ):

            def s_pair(qc, ktp, streams):
                """S^T matmuls for one kt-pair of BOTH streams, emitted so the
                two streams' half-array matmuls (rows 0-63 / 64-127) sit
                adjacent in the PE queue and run concurrently on disjoint row
                groups; then masks + one exp per stream. Returns PT tiles."""
                psss = [
                    ps_s.tile([128, 1024], f32, tag="s", name=f"pss{i}")
                    for i in range(2)
                ]
                for j in (0, 1):
                    kt = 2 * ktp + j
                    for i, (h, hp, ho, psy) in enumerate(streams):
                        nc.tensor.matmul(
                            psss[i][:, j * 512 : (j + 1) * 512],
                            kt_sb[ho : ho + 64, hp, kt * 128 : (kt + 1) * 128],
                            qt_sb[ho : ho + 64, hp, qc * 512 : (qc + 1) * 512],
                            start=True,
                            stop=True,
                        )
                for j in (0, 1):
                    kt = 2 * ktp + j
                    r = kt - 4 * qc
                    if r >= 0:
                        w = 128 * (r + 1)
                        for i in range(2):
                            nc.vector.tensor_tensor(
                                psss[i][:, j * 512 : j * 512 + w],
                                psss[i][:, j * 512 : j * 512 + w],
                                cm_sb[:, 512 - 128 * r : 640],
                                ALU.add,
                            )
                pts = []
                for i in range(2):
                    pt_t = ptp.tile([128, 1024], f32r, tag="pt", name=f"pt{i}")
                    nc.scalar.activation(out=pt_t[:], in_=psss[i][:], func=AF.Exp)
                    pts.append(pt_t)
                return pts

            def pv_block(h, psy, ktp, pt_t, n_kt):
                for j in (0, 1):
                    kt = 2 * ktp + j
                    nc.tensor.matmul(
                        psy[0:65, :],
                        v_sb[:, kt, h * 65 : (h + 1) * 65],
                        pt_t[:, j * 512 : (j + 1) * 512],
                        start=(kt == 0),
                        stop=(kt == n_kt - 1),
                    )

            def normalize(psy, h, hp, ho, qc):
                # copy out of PSUM first so the psy bank frees immediately
                yy = small.tile([65, 512], f32, tag="yy")
                nc.vector.tensor_copy(yy[:], psy[0:65, :])
                scr = dscr.tile([1, 512], f32, tag="scr")
                nc.sync.dma_start(scr[:], yy[64:65, :])
                rb = small.tile([64, 512], f32, tag="rb")
                nc.gpsimd.dma_start(rb[:], scr[:].to_broadcast([64, 512]))
                rc = small.tile([64, 512], f32, tag="rc")
                nc.vector.reciprocal_approx_fast(rc[:], rb[:])
                tmp = small.tile([64, 512], f32r, tag="tmp")
                nc.vector.tensor_tensor(tmp[:], yy[0:64, :], rc[:], ALU.mult)
                nc.sync.dma_start(
                    yt_sb[ho : ho + 64, hp, qc * 512 : (qc + 1) * 512], tmp[:]
                )

            proj_pending = []

            def proj_task(qt):
                """One output tile's projection: 4 full-array matmuls + evac.
                Interleaved into the attention stream as dense PE filler."""
                pso = ps_s.tile([128, 1024], f32, tag="s", name="pso")
                for pc in range(2):
                    for n2 in range(2):
                        nc.tensor.matmul(
                            pso[:, n2 * 512 : (n2 + 1) * 512],
                            yt_sb[:, pc, qt * 128 : (qt + 1) * 128],
                            wp_sb[:, pc, n2 * 512 : (n2 + 1) * 512],
                            start=(pc == 0),
                            stop=(pc == 1),
                        )
                ot = outp.tile([128, C], f32, tag="ot")
                nc.vector.tensor_copy(ot[:], pso[:])
                nc.sync.dma_start(out_d[qt * 128 : (qt + 1) * 128, :], ot[:])

            def attention_group(qc, hh):
                """Two interleaved streams: heads hh (parts 0-63), hh+1 (64-127)."""
                n_kt = 4 * qc + 4
                n_ktp = n_kt // 2
                streams = []
                for h in (hh, hh + 1):
                    hp, ho = h // 2, (h % 2) * 64
                    psy = ps_y.tile([128, 512], f32, tag="y")
                    streams.append((h, hp, ho, psy))
                pts = s_pair(qc, 0, streams)
                for ktp in range(n_ktp):
                    nxt = (
                        s_pair(qc, ktp + 1, streams) if ktp + 1 < n_ktp else None
                    )
                    for i, (h, hp, ho, psy) in enumerate(streams):
                        pv_block(h, psy, ktp, pts[i], n_kt)
                    pts = nxt
                    if ktp % 2 == 1 and proj_pending:
                        proj_task(proj_pending.pop(0))
                for h, hp, ho, psy in streams:
                    normalize(psy, h, hp, ho, qc)

            for qc in range(NT):
                attention_group(qc, 0)
                attention_group(qc, 2)
                proj_pending.extend(range(4 * qc, 4 * qc + 4))
            while proj_pending:
                proj_task(proj_pending.pop(0))

    nc.compile()
    return nc


def make_cmask():
    cm = np.zeros((128, 640), dtype=np.float32)
    cm[:, 0:512] = NEG
    tri = np.zeros((128, 128), dtype=np.float32)
    i = np.arange(128)
    tri[i[:, None] > i[None, :]] = NEG  # rows are k, cols are q: mask k > q
    cm[:, 512:640] = tri
    return cm


def _dev_w(w):
    """[K*128, M] -> [128, K*M]: per-partition contiguous device layout."""
    k = w.shape[0] // 128
    return np.ascontiguousarray(
        w.reshape(k, 128, w.shape[1]).transpose(1, 0, 2).reshape(128, -1)
    ).astype(np.float32)


def make_in_maps(x, w_qkv, b_qkv, w_proj):
    x = np.asarray(x, dtype=np.float32)
    w_qkv = np.asarray(w_qkv, dtype=np.float32)
    b_qkv = np.asarray(b_qkv, dtype=np.float32)
    cmask = make_cmask()
    scale = np.float32(1.0 / np.sqrt(D))
    in_maps = []
    for c in range(N_CORES):
        b, g = divmod(c, 4)
        lo, hi = g * CL, (g + 1) * CL
        in_maps.append(
            dict(
                xt=np.ascontiguousarray(x[b].T),
                wq=_dev_w(w_qkv[:, lo:hi] * scale),
                wk=_dev_w(w_qkv[:, C + lo : C + hi]),
                wv=_dev_w(w_qkv[:, 2 * C + lo : 2 * C + hi]),
                bq=(b_qkv[lo:hi] * scale).reshape(2, 128).copy(),
                bk=b_qkv[C + lo : C + hi].reshape(2, 128).copy(),
                bv=b_qkv[2 * C + lo : 2 * C + hi].copy(),
                wp=_dev_w(np.asarray(w_proj, dtype=np.float32)[lo:hi, :]),
                cmask=cmask,
                vones=np.ones(1, dtype=np.float32),
            )
        )
    return in_maps


_CACHED_NC = None


def _get_nc():
    global _CACHED_NC
    if _CACHED_NC is None:
        _CACHED_NC = build_bass()
    return _CACHED_NC


def kernel(x, w_qkv, b_qkv, w_proj, b_proj):
    from concourse.bass_utils import run_bass_kernel_spmd

    nc = _get_nc()
    in_maps = make_in_maps(x, w_qkv, b_qkv, w_proj)
    res = run_bass_kernel_spmd(nc, in_maps, core_ids=list(range(N_CORES)))
    parts = [res.results[c]["out"] for c in range(N_CORES)]
    b_proj = np.asarray(b_proj, dtype=np.float32)
    out = np.stack(
        [
            parts[0] + parts[1] + parts[2] + parts[3],
            parts[4] + parts[5] + parts[6] + parts[7],
        ],
        axis=0,
    )
    return (out + b_proj).astype(np.float32)


# revision 30
# speedup vs baseline: 1.3867x; 1.3867x over previous
"""Causal self-attention (B=2, T=2048, C=1024, H=16) on 8 trn2 NeuronCores.

Sharding: data-parallel over batch (2) x tensor-parallel over head groups (4);
each core handles one (batch, 4-head group) pair end to end: qkv projection
(column-sliced weights) -> causal attention -> proj with row-sliced w_proj
producing a partial [T, C] output; host sums 4 partials per batch + b_proj.

Device scheme (all matmul operands float32r; PSUM accumulates fp32):
  xt   [C, T]       x[b] transposed (host); moving operand for Q^T/K^T,
                    stationary for V.
  Q^T/K^T [128,2,T] head-dim on partitions, 2 heads per 128-partition tile;
                    softmax 1/sqrt(D) folded into wq/bq on host.
  V'   [128,16,4*65] per k-tile: 64 value cols + a ones column per head
                    (P@[V|1] row 64 = softmax denominator).
  S^T  [128k, 512q] scores in PSUM, kt-pairs share one [128,1024] tile so a
                    single exp covers two k-tiles; causal = block skip above
                    the diagonal + additive -1e30 mask on diagonal blocks;
                    no max subtraction (scores ~N(0,1)).
  y'^T [65, 512]    accumulated over k-tiles; normalized by the denominator
                    row via fast reciprocal + DRAM-bounce broadcast.

Perf structure: QKV runs kc-outer over 8 concurrent PSUM groups so matmuls
start as soon as the first xt/w chunks land; attention interleaves two head
streams (partitions 0-63 and 64-127) so the PE never idles while ACT does
exp (keeps the HAM clock gate warm); proj for chunk qc is emitted one qc
late so the normalization chain is off the critical path.
"""

import numpy as np

N_CORES = 8
B, T, C = 2, 2048, 1024
H, D = 16, 64
HL = 4            # heads per core
CL = HL * D       # 256 local qkv columns per core
KC = C // 128     # 8 contraction chunks
NT = T // 512     # 4 free-dim chunks
TT = T // 128     # 16 token tiles
NEG = -1.0e30


def build_bass():
    from contextlib import ExitStack

    import concourse.mybir as mybir
    import concourse.tile as tile
    from concourse import bacc

    f32 = mybir.dt.float32
    f32r = mybir.dt.float32r
    AF = mybir.ActivationFunctionType
    ALU = mybir.AluOpType

    nc = bacc.Bacc("TRN2", target_bir_lowering=False, debug=False)

    xt_d = nc.dram_tensor("xt", [C, T], f32, kind="ExternalInput").ap()
    wq_d = nc.dram_tensor("wq", [128, KC * CL], f32, kind="ExternalInput").ap()
    wk_d = nc.dram_tensor("wk", [128, KC * CL], f32, kind="ExternalInput").ap()
    wv_d = nc.dram_tensor("wv", [128, KC * CL], f32, kind="ExternalInput").ap()
    bq_d = nc.dram_tensor("bq", [2, 128], f32, kind="ExternalInput").ap()
    bk_d = nc.dram_tensor("bk", [2, 128], f32, kind="ExternalInput").ap()
    bv_d = nc.dram_tensor("bv", [CL], f32, kind="ExternalInput").ap()
    wp_d = nc.dram_tensor("wp", [128, 2 * C], f32, kind="ExternalInput").ap()
    cm_d = nc.dram_tensor("cmask", [128, 640], f32, kind="ExternalInput").ap()
    vones_d = nc.dram_tensor("vones", [128, TT * HL], f32, kind="ExternalInput").ap()
    out_d = nc.dram_tensor("out", [T, C], f32, kind="ExternalOutput").ap()

    with tile.TileContext(nc) as tc, ExitStack() as ctx:
        singles = ctx.enter_context(tc.tile_pool(name="singles", bufs=1))
        ptp = ctx.enter_context(tc.tile_pool(name="ptp", bufs=3))
        small = ctx.enter_context(tc.tile_pool(name="small", bufs=2))
        outp = ctx.enter_context(tc.tile_pool(name="outp", bufs=2))
        dscr = ctx.enter_context(tc.tile_pool(name="dscr", bufs=3, space="DRAM"))

        # small constants first, on the gpsimd (SWDGE) queue
        bq_sb = singles.tile([128, 2], f32)
        nc.gpsimd.dma_start(bq_sb[:], bq_d.rearrange("pt p -> p pt"))
        bk_sb = singles.tile([128, 2], f32)
        nc.gpsimd.dma_start(bk_sb[:], bk_d.rearrange("pt p -> p pt"))
        bv_sb = singles.tile([128, CL], f32)
        nc.gpsimd.dma_start(bv_sb[:], bv_d[None, :].to_broadcast([128, CL]))
        cm_sb = singles.tile([128, 640], f32)
        nc.gpsimd.dma_start(cm_sb[:], cm_d[:])

        wq_sb = singles.tile([128, KC, CL], f32r)
        wk_sb = singles.tile([128, KC, CL], f32r)
        wv_sb = singles.tile([128, KC, CL], f32r)
        wp_sb = singles.tile([128, 2, C], f32r)

        qt_sb = singles.tile([128, 2, T], f32r)
        kt_sb = singles.tile([128, 2, T], f32r)
        v_sb = singles.tile([128, TT, HL * 65], f32r)
        yt_sb = singles.tile([128, 2, T], f32r)

        # ones column per (k-tile, head) for the denominator trick: load a
        # contiguous ones block, then one strided DVE copy (a broadcast DMA
        # here would emit 8k tiny descriptors and clog the shared DMA ports)
        ones_sb = singles.tile([128, TT * HL], f32)
        nc.gpsimd.dma_start(ones_sb[:], vones_d[:])
        v_ones = v_sb[:].rearrange("p t (h e) -> p (t h) e", e=65)[:, :, 64:65]
        nc.vector.tensor_copy(
            v_ones, ones_sb[:].rearrange("p (a b) -> p a b", b=1)
        )

        # ---- Phase A+B: qkv projections (xt resident only here) ----
        with tc.tile_pool(name="xtp", bufs=1) as xtp:
            xt_sb = xtp.tile([128, KC, T], f32r)
            # per-chunk loads, interleaved across both HWDGE queues so the
            # kc-outer matmul passes can start after the first chunks land
            # all big input loads on the sync queue only: one HWDGE queue
            # reaches full rate, and the scalar queue must stay free for the
            # psum-evacuation ACTIVATEs that recycle the qkv psum slots
            nc.sync.dma_start(
                wq_sb[:], wq_d.rearrange("p (kc m) -> p kc m", kc=KC).bitcast(f32r)
            )
            nc.sync.dma_start(
                wk_sb[:], wk_d.rearrange("p (kc m) -> p kc m", kc=KC).bitcast(f32r)
            )
            nc.sync.dma_start(
                wv_sb[:], wv_d.rearrange("p (kc m) -> p kc m", kc=KC).bitcast(f32r)
            )
            for kc in range(KC):
                nc.sync.dma_start(
                    xt_sb[:, kc, :],
                    xt_d[kc * 128 : (kc + 1) * 128, :].bitcast(f32r),
                )
            nc.sync.dma_start(
                wp_sb[:], wp_d.rearrange("p (pc n) -> p pc n", pc=2).bitcast(f32r)
            )

            groups = [(pt, nt) for pt in range(2) for nt in range(NT)]
            with tc.tile_pool(name="ps8", bufs=8, space="PSUM") as ps8:
                for w_sb, b_sb, dst in (
                    (wq_sb, bq_sb, qt_sb),
                    (wk_sb, bk_sb, kt_sb),
                ):
                    tiles = [
                        ps8.tile([128, 512], f32, tag="qk", name=f"qkg{g}")
                        for g in range(len(groups))
                    ]
                    for kc in range(KC):
                        for g, (pt, nt) in enumerate(groups):
                            nc.tensor.matmul(
                                tiles[g][:],
                                w_sb[:, kc, pt * 128 : (pt + 1) * 128],
                                xt_sb[:, kc, nt * 512 : (nt + 1) * 512],
                                start=(kc == 0),
                                stop=(kc == KC - 1),
                            )
                    for g, (pt, nt) in enumerate(groups):
                        nc.scalar.activation(
                            out=dst[:, pt, nt * 512 : (nt + 1) * 512],
                            in_=tiles[g][:],
                            func=AF.Identity,
                            bias=b_sb[:, pt : pt + 1],
                            scale=1.0,
                        )

            with tc.tile_pool(name="ps_v", bufs=2, space="PSUM") as ps_v:
                for tt in range(TT):
                    ps = ps_v.tile([128, CL], f32, tag="v")
                    for kc in range(KC):
                        nc.tensor.matmul(
                            ps[:],
                            xt_sb[:, kc, tt * 128 : (tt + 1) * 128],
                            wv_sb[:, kc, :],
                            start=(kc == 0),
                            stop=(kc == KC - 1),
                        )
                    dst = v_sb[:, tt, :].rearrange("p (h e) -> p h e", e=65)[
                        :, :, 0:64
                    ]
                    nc.vector.tensor_tensor(
                        dst,
                        ps[:].rearrange("p (h e) -> p h e", e=64),
                        bv_sb[:].rearrange("p (h e) -> p h e", e=64),
                        ALU.add,
                    )

        # ---- Phase C+D: attention (two interleaved head streams) + proj ----
        with (
            tc.tile_pool(name="ps_s", bufs=3, space="PSUM") as ps_s,
            tc.tile_pool(name="ps_y", bufs=2, space="PSUM") as ps_y,
        ):

            def s_pair(qc, ktp, streams):
                """S^T matmuls for one kt-pair of BOTH streams, emitted so the
                two streams' half-array matmuls (rows 0-63 / 64-127) sit
                adjacent in the PE queue and run concurrently on disjoint row
                groups; then masks + one exp per stream. Returns PT tiles."""
                psss = [
                    ps_s.tile([128, 1024], f32, tag="s", name=f"pss{i}")
                    for i in range(2)
                ]
                for j in (0, 1):
                    kt = 2 * ktp + j
                    for i, (h, hp, ho, psy) in enumerate(streams):
                        nc.tensor.matmul(
                            psss[i][:, j * 512 : (j + 1) * 512],
                            kt_sb[ho : ho + 64, hp, kt * 128 : (kt + 1) * 128],
                            qt_sb[ho : ho + 64, hp, qc * 512 : (qc + 1) * 512],
                            start=True,
                            stop=True,
                        )
                for j in (0, 1):
                    kt = 2 * ktp + j
                    r = kt - 4 * qc
                    if r >= 0:
                        w = 128 * (r + 1)
                        for i in range(2):
                            nc.vector.tensor_tensor(
                                psss[i][:, j * 512 : j * 512 + w],
                                psss[i][:, j * 512 : j * 512 + w],
                                cm_sb[:, 512 - 128 * r : 640],
                                ALU.add,
                            )
                pts = []
                for i in range(2):
                    pt_t = ptp.tile([128, 1024], f32r, tag="pt", name=f"pt{i}")
                    nc.scalar.activation(out=pt_t[:], in_=psss[i][:], func=AF.Exp)
                    pts.append(pt_t)
                return pts

            def pv_block(h, psy, ktp, pt_t, n_kt):
                for j in (0, 1):
                    kt = 2 * ktp + j
                    nc.tensor.matmul(
                        psy[0:65, :],
                        v_sb[:, kt, h * 65 : (h + 1) * 65],
                        pt_t[:, j * 512 : (j + 1) * 512],
                        start=(kt == 0),
                        stop=(kt == n_kt - 1),
                    )

            def normalize(psy, h, hp, ho, qc):
                # copy out of PSUM first so the psy bank frees immediately
                yy = small.tile([65, 512], f32, tag="yy")
                nc.vector.tensor_copy(yy[:], psy[0:65, :])
                scr = dscr.tile([1, 512], f32, tag="scr")
                nc.sync.dma_start(scr[:], yy[64:65, :])
                rb = small.tile([64, 512], f32, tag="rb")
                nc.gpsimd.dma_start(rb[:], scr[:].to_broadcast([64, 512]))
                rc = small.tile([64, 512], f32, tag="rc")
                nc.vector.reciprocal_approx_fast(rc[:], rb[:])
                tmp = small.tile([64, 512], f32r, tag="tmp")
                nc.vector.tensor_tensor(tmp[:], yy[0:64, :], rc[:], ALU.mult)
                nc.sync.dma_start(
                    yt_sb[ho : ho + 64, hp, qc * 512 : (qc + 1) * 512], tmp[:]
                )

            proj_pending = []

            def proj_task(qt):
                """One output tile's projection: 4 full-array matmuls + evac.
                Interleaved into the attention stream as dense PE filler."""
                pso = ps_s.tile([128, 1024], f32, tag="s", name="pso")
                for pc in range(2):
                    for n2 in range(2):
                        nc.tensor.matmul(
                            pso[:, n2 * 512 : (n2 + 1) * 512],
                            yt_sb[:, pc, qt * 128 : (qt + 1) * 128],
                            wp_sb[:, pc, n2 * 512 : (n2 + 1) * 512],
                            start=(pc == 0),
                            stop=(pc == 1),
                        )
                ot = outp.tile([128, C], f32, tag="ot")
                nc.vector.tensor_copy(ot[:], pso[:])
                nc.sync.dma_start(out_d[qt * 128 : (qt + 1) * 128, :], ot[:])

            def attention_group(qc, hh):
                """Two interleaved streams: heads hh (parts 0-63), hh+1 (64-127)."""
                n_kt = 4 * qc + 4
                n_ktp = n_kt // 2
                streams = []
                for h in (hh, hh + 1):
                    hp, ho = h // 2, (h % 2) * 64
                    psy = ps_y.tile([128, 512], f32, tag="y")
                    streams.append((h, hp, ho, psy))
                pts = s_pair(qc, 0, streams)
                for ktp in range(n_ktp):
                    nxt = (
                        s_pair(qc, ktp + 1, streams) if ktp + 1 < n_ktp else None
                    )
                    for i, (h, hp, ho, psy) in enumerate(streams):
                        pv_block(h, psy, ktp, pts[i], n_kt)
                    pts = nxt
                    if ktp % 2 == 1 and proj_pending:
                        proj_task(proj_pending.pop(0))
                for h, hp, ho, psy in streams:
                    normalize(psy, h, hp, ho, qc)

            for qc in range(NT):
                attention_group(qc, 0)
                attention_group(qc, 2)
                proj_pending.extend(range(4 * qc, 4 * qc + 4))
            while proj_pending:
                proj_task(proj_pending.pop(0))

    nc.compile()
    return nc


def make_cmask():
    cm = np.zeros((128, 640), dtype=np.float32)
    cm[:, 0:512] = NEG
    tri = np.zeros((128, 128), dtype=np.float32)
    i = np.arange(128)
    tri[i[:, None] > i[None, :]] = NEG  # rows are k, cols are q: mask k > q
    cm[:, 512:640] = tri
    return cm


def _dev_w(w):
    """[K*128, M] -> [128, K*M]: per-partition contiguous device layout."""
    k = w.shape[0] // 128
    return np.ascontiguousarray(
        w.reshape(k, 128, w.shape[1]).transpose(1, 0, 2).reshape(128, -1)
    ).astype(np.float32)


def make_in_maps(x, w_qkv, b_qkv, w_proj):
    x = np.asarray(x, dtype=np.float32)
    w_qkv = np.asarray(w_qkv, dtype=np.float32)
    b_qkv = np.asarray(b_qkv, dtype=np.float32)
    cmask = make_cmask()
    scale = np.float32(1.0 / np.sqrt(D))
    in_maps = []
    for c in range(N_CORES):
        b, g = divmod(c, 4)
        lo, hi = g * CL, (g + 1) * CL
        in_maps.append(
            dict(
                xt=np.ascontiguousarray(x[b].T),
                wq=_dev_w(w_qkv[:, lo:hi] * scale),
                wk=_dev_w(w_qkv[:, C + lo : C + hi]),
                wv=_dev_w(w_qkv[:, 2 * C + lo : 2 * C + hi]),
                bq=(b_qkv[lo:hi] * scale).reshape(2, 128).copy(),
                bk=b_qkv[C + lo : C + hi].reshape(2, 128).copy(),
                bv=b_qkv[2 * C + lo : 2 * C + hi].copy(),
                wp=_dev_w(np.asarray(w_proj, dtype=np.float32)[lo:hi, :]),
                cmask=cmask,
                vones=np.ones((128, TT * HL), dtype=np.float32),
            )
        )
    return in_maps


_CACHED_NC = None


def _get_nc():
    global _CACHED_NC
    if _CACHED_NC is None:
        _CACHED_NC = build_bass()
    return _CACHED_NC


def kernel(x, w_qkv, b_qkv, w_proj, b_proj):
    from concourse.bass_utils import run_bass_kernel_spmd

    nc = _get_nc()
    in_maps = make_in_maps(x, w_qkv, b_qkv, w_proj)
    res = run_bass_kernel_spmd(nc, in_maps, core_ids=list(range(N_CORES)))
    parts = [res.results[c]["out"] for c in range(N_CORES)]
    b_proj = np.asarray(b_proj, dtype=np.float32)
    out = np.stack(
        [
            parts[0] + parts[1] + parts[2] + parts[3],
            parts[4] + parts[5] + parts[6] + parts[7],
        ],
        axis=0,
    )
    return (out + b_proj).astype(np.float32)
